# revision 7
# baseline (speedup 1.0000x reference)
"""Bass/Tile multi-head attention kernel for TRN2.

Per-core problem (core c handles batch b=c//2, head-group g=c%2):
  inputs:  xq, xk, xv [S, DIN] bf16     (batch b slices of q/k/v, host-cast)
           wq, wk, wv [DIN, DC] bf16    (column slice for this head group)
           wo [DC, DOUT] bf16           (row slice)
           bq, bk, bv [DC] f32
  output:  out [S, DOUT] bf16  partial: host sums the two head-group
           partials per batch in f32 and adds bo.

Math (per head h of H local heads, depth=64):
  QT = (xq @ wq + bq).T        [DC, S]   f32r, d_core major
  KT = (xk @ wk + bk).T        [DC, S]   f32r
  V  = xv @ wv + bv            [S, DC]   bf16 (+ ones column -> V_aug)
  ST_h = KT_h.T @ QT_h         (64-deep contraction at partition base
                                (h%2)*64 -- no zero-padded Q copies)
  E = exp(ST * 1/sqrt(depth))            (logits are O(6): no row-max pass)
  OT_aug = V_aug_h.T @ E       [65, sq]  (row 64 = softmax denominator)
  OTn_h = OT_aug[0:64] / OT_aug[64]      bf16
  out = OTn.T @ wo                       (bf16 x bf16 -> f32 psum)

Engine budget (cost model): PE ~327us (binding: proj/ST/AV at ~109us each,
ZERO transposes), ACT ~266us exp-only, DVE ~120us, DMA ~118us incl. XBAR
transposes.  Design rules:
 - host pre-casts x and weights to bf16; x^T comes from XBAR DMA-transposes
   (dma_start_transpose, 2-byte only, 16x128 tiles at ~14ns/tile) straight
   from DRAM into SBUF -- no PE transposes, no PSUM staging, no copies.
   Each 512-row chunk is two [512, 512] transposes so the first matmul
   k-steps start after half a chunk.  Attention QK^T stays f32r
   (rel err ~6e-3 vs the 2e-2 budget).
 - queue assignment: x transposes on sync/SP (hwdge), weights as two
   half-tensor DMAs on the Activation hwdge queue, out partials + biases
   on the gpsimd (swdge) queue -- except the last tile's outputs, which
   go on sync so the ~1us swdge generation never lands in the drain tail.
 - PSUM: st 2x2 banks + ot 2x1 + gen 2x1 = 8, all double-buffered.
 - AV matmuls trail the STs by `trail` kg-groups ACROSS head boundaries,
   and each sqt's out-projection is emitted inside the NEXT sqt's loop,
   so PE never waits on exp / the final norm chain; the last head of the
   last tile normalizes per 128-column chunk to un-gate the final
   out-projection sooner.
 - ACT runs softmax exp only; bias adds + normalization on DVE/Pool.

NOTE program order is load-bearing: every tile's writer must be EMITTED
before its first reader (the tile framework treats emission order as
happens-before; a reader emitted first reads garbage on hw).
"""

from contextlib import ExitStack

import concourse.mybir as mybir
from concourse import bacc
from concourse.tile import TileContext

F32 = mybir.dt.float32
F32R = mybir.dt.float32r
BF16 = mybir.dt.bfloat16
P = 128
EXP = mybir.ActivationFunctionType.Exp


def build_mha_core(S=2048, DIN=1024, DC=512, DOUT=1024, H=8, depth=64,
                   SQT=512, KG=2, num_devices=1, trail=28, stage_bufs=6,
                   ex_bufs=29, kbias_act=False, stage_rows=128, opos=32, qh=3):
    assert DC == H * depth and DC % P == 0 and DIN % P == 0 and S % SQT == 0
    NKT = S // P          # key chunks of 128
    NDIN = DIN // P       # input-dim k-tiles
    NDO = DC // P         # d_core blocks
    NSQT = S // SQT       # attention q tiles
    NKG = NKT // KG       # kg groups per head
    NCH = S // 512        # 512-row x chunks
    scale = 1.0 / float(depth) ** 0.5

    nc = bacc.Bacc("TRN2", target_bir_lowering=False, debug=False,
                   num_devices=num_devices)
    xq = nc.dram_tensor("xq", [S, DIN], BF16, kind="ExternalInput")
    xk = nc.dram_tensor("xk", [S, DIN], BF16, kind="ExternalInput")
    xv = nc.dram_tensor("xv", [S, DIN], BF16, kind="ExternalInput")
    wq = nc.dram_tensor("wq", [DIN, DC], BF16, kind="ExternalInput")
    wk = nc.dram_tensor("wk", [DIN, DC], BF16, kind="ExternalInput")
    wv = nc.dram_tensor("wv", [DIN, DC], BF16, kind="ExternalInput")
    wo = nc.dram_tensor("wo", [DC, DOUT], BF16, kind="ExternalInput")
    bq = nc.dram_tensor("bq", [DC], F32, kind="ExternalInput")
    bk = nc.dram_tensor("bk", [DC], F32, kind="ExternalInput")
    bv = nc.dram_tensor("bv", [DC], F32, kind="ExternalInput")
    out = nc.dram_tensor("out", [S, DOUT], BF16, kind="ExternalOutput")

    with TileContext(nc) as tc, ExitStack() as ctx:
        const = ctx.enter_context(tc.tile_pool(name="const", bufs=1))
        wqpool = ctx.enter_context(tc.tile_pool(name="wqp", bufs=1))
        kvpool = ctx.enter_context(tc.tile_pool(name="kv", bufs=1))
        stage = ctx.enter_context(tc.tile_pool(name="stage", bufs=stage_bufs))
        xtpool = ctx.enter_context(tc.tile_pool(name="xt", bufs=3))
        qpool = ctx.enter_context(tc.tile_pool(name="qp", bufs=2))
        ps_st = ctx.enter_context(tc.tile_pool(name="ps_st", bufs=2, space="PSUM"))
        ps_ot = ctx.enter_context(tc.tile_pool(name="ps_ot", bufs=2, space="PSUM"))
        ps_gen = ctx.enter_context(tc.tile_pool(name="ps_gen", bufs=2, space="PSUM"))

        ones_f = const.tile([P, 1], F32)
        nc.vector.memset(ones_f[:], 1.0)

        # weights + biases go through the (otherwise idle) gpsimd DMA queue
        # so they don't serialize behind the x staging DMAs on the sync queue
        bq_sb = const.tile([P, NDO], F32)
        bk_sb = const.tile([P, NDO], F32)
        bv_st = const.tile([1, DC], F32)
        bv_bc = const.tile([P, DC], F32)

        def load_biases():
            # bk/bv only: their first consumers (kproj/vproj bias adds) are
            # emitted after this point.  bq must load before qproj(0).
            nc.gpsimd.dma_start(bk_sb[:],
                                bk[:].rearrange("(o p) -> p o", p=P))
            nc.gpsimd.dma_start(bv_st[0:1, :], bv[:][None, :])
            nc.gpsimd.partition_broadcast(bv_bc[:], bv_st[0:1, :])

        KT = kvpool.tile([P, NDO, S], F32R)
        V = kvpool.tile([P, NKT, H, depth + 1], BF16)
        nc.vector.tensor_copy(
            V[:, :, :, depth:depth + 1],
            ones_f[:, None, None, 0:1].to_broadcast((P, NKT, H, 1)))

        def load_weight(pool, dram, kdim, ndim, tag, act_only=False):
            # halves on parallel queues (ACT hwdge + gpsimd swdge) so both
            # land ~together instead of serializing on one queue.  act_only
            # keeps both halves on ACT: at kernel start the gpsimd swdge
            # half loses the DMA_ENGINES fifo race to the (much later
            # needed) xkT transposes and arrives ~5us late.
            w = pool.tile([P, kdim // P, ndim], BF16, tag=tag, name=tag)
            half = kdim // P // 2
            engs = (nc.scalar, nc.scalar) if act_only else (nc.scalar,
                                                            nc.gpsimd)
            for g, eng in enumerate(engs):
                eng.dma_start(
                    w[:, g * half:(g + 1) * half, :],
                    dram[g * half * P:(g + 1) * half * P, :]
                    .rearrange("(o p) n -> p o n", p=P))
            return w

        # ---- x transposition: 512 rows of xdram -> xt [P, NDIN, 512] ----
        # One XBAR DMA-transpose (bf16, 16x128 tiles):
        # xt[p, o, s] = x[c*512+s, o*128+p].  No PE transposes, no staging.
        def make_xt(xdram, c, kv_phase=False, dma=None):
            dma = dma or nc.sync
            xt = xtpool.tile([P, NDIN, 512], BF16, tag="xt", name="xt")
            hd = NDIN // 2
            for g in range(2):  # split by din so the first matmuls start early
                dma.dma_start_transpose(
                    xt[:, g * hd:(g + 1) * hd, :],
                    xdram[c * 512:(c + 1) * 512,
                          g * hd * P:(g + 1) * hd * P])
            return xt

        def qproj(sqt, xt=None):
            if xt is None:
                xt = make_xt(xq, sqt, kv_phase=False)
            QT = qpool.tile([P, NDO, SQT], F32R, tag="qt", name="qt")
            for do in range(NDO):
                ps = ps_gen.tile([P, 512], F32, tag="gen", name="psq")
                for kt in range(NDIN):
                    nc.tensor.matmul(
                        ps[:, :SQT], wqr[:, kt, do * P:(do + 1) * P],
                        xt[:, kt, :], start=(kt == 0), stop=(kt == NDIN - 1))
                nc.vector.tensor_scalar_add(QT[:, do, :], ps[:, :SQT],
                                            bq_sb[:, do:do + 1])
            return QT

        xt_q0 = make_xt(xq, 0, kv_phase=True)
        nc.gpsimd.dma_start(bq_sb[:], bq[:].rearrange("(o p) -> p o", p=P))
        wqr = load_weight(wqpool, wq, DIN, DC, "wq", act_only=True)
        QT_next = qproj(0, xt=xt_q0)

        # ---- K/V production (chunked; PE-bound, ACT+Pool+DVE assist) ----
        with tc.tile_pool(name="wkv", bufs=1) as wkvpool:
            wkr = wvr = None
            for c in range(NCH):
                xkt = make_xt(xk, c, kv_phase=True)
                if wkr is None:
                    wkr = load_weight(wkvpool, wk, DIN, DC, "wk")
                    load_biases()
                for do in range(NDO):
                    ps = ps_gen.tile([P, 512], F32, tag="gen", name="psk")
                    for kt in range(NDIN):
                        nc.tensor.matmul(
                            ps[:], wkr[:, kt, do * P:(do + 1) * P],
                            xkt[:, kt, :], start=(kt == 0),
                            stop=(kt == NDIN - 1))
                    if kbias_act:
                        nc.scalar.activation(
                            KT[:, do, c * 512:(c + 1) * 512], ps[:],
                            mybir.ActivationFunctionType.Identity,
                            bias=bk_sb[:, do:do + 1])
                    else:
                        nc.vector.tensor_scalar_add(
                            KT[:, do, c * 512:(c + 1) * 512], ps[:],
                            bk_sb[:, do:do + 1])
                xvt = make_xt(xv, c, kv_phase=True)
                if wvr is None:
                    wvr = load_weight(wkvpool, wv, DIN, DC, "wv")
                for sc in range(4):
                    ps = ps_gen.tile([P, 512], F32, tag="gen", name="psv")
                    for kt in range(NDIN):
                        nc.tensor.matmul(
                            ps[:], xvt[:, kt, sc * P:(sc + 1) * P],
                            wvr[:, kt, :], start=(kt == 0),
                            stop=(kt == NDIN - 1))
                    chunk = c * 4 + sc
                    nc.vector.tensor_tensor(
                        V[:, chunk, :, 0:depth],
                        ps[:].rearrange("p (h d) -> p h d", h=H),
                        bv_bc[:].rearrange("p (h d) -> p h d", h=H),
                        mybir.AluOpType.add)

        wor = load_weight(wqpool, wo, DC, DOUT, "wo")

        # ---- attention + out-projection ----
        expool = ctx.enter_context(tc.tile_pool(name="ex", bufs=ex_bufs))
        otT_pool = ctx.enter_context(tc.tile_pool(name="otnt", bufs=2))
        otq_pool = ctx.enter_context(tc.tile_pool(name="otq", bufs=2))
        osbpool = ctx.enter_context(tc.tile_pool(name="osb", bufs=3))
        misc = ctx.enter_context(tc.tile_pool(name="misc", bufs=2))

        NQC = SQT // P  # 128-query chunks per q tile

        def norm_head(h, ot, otnq, OTnT, fine=False):
            # Swapped-AV normalization: ot is [q=128, NQC, depth+1] PSUM with
            # the softmax denominator in column `depth` -- a per-partition
            # scalar, so normalization is reciprocal + broadcast-mult on DVE
            # (no Pool partition_broadcast).  Results go into a per-PAIR
            # [q, NQC, 128] bf16 tile; once both heads of the pair have
            # landed, each 128x128 q-chunk is XBAR-transposed (SBUF->SBUF DMA,
            # zero engine cost) into OTnT[pair] = [dc, SQT], the layout the
            # out-projection consumes.
            pair = h // 2
            p0 = (h % 2) * 64
            if h % 2 == 0:
                otnq[pair] = otq_pool.tile([P, NQC, P], BF16, tag="otq",
                                           name="otq")
            q_tile = otnq[pair]
            if fine:
                # last head of the last tile: per-qc chain so each
                # out-projection chunk un-gates as early as possible
                for qc in range(NQC):
                    rec = misc.tile([P, 1, 1], F32, tag="recf", name="recf")
                    nc.vector.reciprocal(
                        rec[:], ot[:, qc:qc + 1, depth:depth + 1])
                    nc.vector.tensor_tensor(
                        q_tile[:, qc:qc + 1, p0:p0 + depth],
                        ot[:, qc:qc + 1, 0:depth],
                        rec[:, :, 0:1].to_broadcast((P, 1, depth)),
                        mybir.AluOpType.mult)
                    nc.sync.dma_start_transpose(
                        OTnT[pair][:, qc * P:(qc + 1) * P], q_tile[:, qc, :])
                return
            rec = misc.tile([P, NQC, 1], F32, tag="rec", name="rec")
            nc.vector.reciprocal(rec[:], ot[:, :, depth:depth + 1])
            nc.vector.tensor_tensor(
                q_tile[:, :, p0:p0 + depth], ot[:, :, 0:depth],
                rec[:, :, 0:1].to_broadcast((P, NQC, depth)),
                mybir.AluOpType.mult)
            if h % 2 == 1:
                for qc in range(NQC):
                    nc.sync.dma_start_transpose(
                        OTnT[pair][:, qc * P:(qc + 1) * P], q_tile[:, qc, :])

        def do_oproj(OTnT, sqt, copy_act=False, out_sync=False):
            for sc in range(SQT // P):
                osb = osbpool.tile([P, DOUT], BF16, tag="osb", name="osb")
                r0 = sqt * SQT + sc * P
                for do in range(DOUT // 512):
                    ps = ps_gen.tile([P, 512], F32, tag="gen", name="pso")
                    for i in range(NDO):
                        nc.tensor.matmul(
                            ps[:], OTnT[i][:, sc * P:(sc + 1) * P],
                            wor[:, i, do * 512:(do + 1) * 512],
                            start=(i == 0), stop=(i == NDO - 1))
                    if copy_act:
                        nc.scalar.copy(osb[:, do * 512:(do + 1) * 512], ps[:])
                    else:
                        nc.vector.tensor_copy(
                            osb[:, do * 512:(do + 1) * 512], ps[:])
                    (nc.sync if out_sync else nc.gpsimd).dma_start(
                        out[r0:r0 + P, do * 512:(do + 1) * 512],
                        osb[:, do * 512:(do + 1) * 512])

        prev_otnT = None
        for sqt in range(NSQT):
            QT = QT_next
            OTnT = [otT_pool.tile([P, SQT], BF16, tag=f"otnt{blk}",
                                  name="otnt") for blk in range(NDO)]
            ots, exs, otnq = {}, {}, {}

            def st_step(h, kg):
                p0, blk = (h % 2) * 64, h // 2
                st = ps_st.tile([P, KG, 512], F32, tag="st", name="st")
                for j in range(KG):
                    kt = kg * KG + j
                    nc.tensor.matmul(
                        st[:, j], KT[p0:p0 + 64, blk, kt * P:(kt + 1) * P],
                        QT[p0:p0 + 64, blk, :], start=True, stop=True)
                ex = expool.tile([P, KG, 512], BF16, tag="ex", name="ex")
                exs[(h, kg)] = ex
                nc.scalar.activation(ex[:], st[:], EXP, scale=scale)

            def av_step(h, kg):
                # swapped orientation: O[q, d] = sum_k ex[k, q] V[k, d] with
                # the 128x128 ex block as PE stationary and the [128, 65]
                # V_aug chunk as moving -- 65-row matmuls, full 128-deep
                # contraction, half the PE time of the [65, q] orientation.
                if kg == 0:
                    # allocated here (not at ST time) so the ot-pool rotation
                    # follows AV order and never throttles the ST stream
                    ots[h] = ps_ot.tile([P, NQC, P], F32, tag="ot",
                                        name="ot")
                ex = exs.pop((h, kg))
                ot = ots[h]
                for j in range(KG):
                    kt = kg * KG + j
                    for qc in range(NQC):
                        # start/stop once per BANK: start_tensor_calc marks
                        # the whole 2KB zero region pending-zero, so the
                        # first write of each qc sub-region auto-overwrites
                        nc.tensor.matmul(
                            ot[:, qc, 0:depth + 1],
                            ex[:, j, qc * P:(qc + 1) * P],
                            V[:, kt, h, :],
                            start=(kt == 0 and qc == 0),
                            stop=(kt == NKT - 1 and qc == NQC - 1))
                if kg == NKG - 1:
                    norm_head(h, ots.pop(h), otnq, OTnT,
                              fine=(sqt == NSQT - 1 and h == H - 1))
                    if h == qh and sqt + 1 < NSQT:
                        nonlocal_qt[0] = qproj(sqt + 1)

            nonlocal_qt = [None]
            trail_eff = trail
            steps = [(h, kg) for h in range(H) for kg in range(NKG)]
            for i, (h, kg) in enumerate(steps):
                st_step(h, kg)
                if i == opos and prev_otnT is not None:
                    # previous sqt's out-projection, emitted here so its last
                    # accumulation step never stalls the PE (the last norm of
                    # that sqt has long drained by now)
                    do_oproj(prev_otnT, sqt - 1)
                if i >= trail_eff:
                    av_step(*steps[i - trail_eff])
            for i in range(len(steps) - trail_eff, len(steps)):
                av_step(*steps[i])
            if nonlocal_qt[0] is not None:
                QT_next = nonlocal_qt[0]
            prev_otnT = OTnT

        do_oproj(prev_otnT, NSQT - 1, copy_act=True, out_sync=True)

    nc.compile()
    return nc


# ---------------------------------------------------------------------------
# Host-side wrapper: shard across 8 NeuronCores, run SPMD, gather.
# Core c handles batch b = c // 2 and head-group g = c % 2 (8 of 16 heads,
# i.e. columns [g*512, (g+1)*512) of Wq/Wk/Wv and rows of Wo).
# ---------------------------------------------------------------------------

import ml_dtypes
import numpy as np

from concourse.bass_utils import run_bass_kernel_spmd

_BF16 = ml_dtypes.bfloat16

_NC = None


def _get_nc():
    global _NC
    if _NC is None:
        _NC = build_mha_core(S=2048, DIN=1024, DC=512, DOUT=1024, H=8,
                             depth=64, num_devices=8)
    return _NC


def _in_maps(q, k, v, Wq, bq, Wk, bk, Wv, bv, Wo, bo):
    f32 = np.float32
    qb = np.asarray(q, dtype=_BF16)
    kb = np.asarray(k, dtype=_BF16)
    vb = np.asarray(v, dtype=_BF16)
    Wqb = np.asarray(Wq, dtype=_BF16)
    Wkb = np.asarray(Wk, dtype=_BF16)
    Wvb = np.asarray(Wv, dtype=_BF16)
    Wob = np.asarray(Wo, dtype=_BF16)
    maps = []
    for c in range(8):
        b, g = c // 2, c % 2
        sl = slice(g * 512, (g + 1) * 512)
        maps.append({
            "xq": np.ascontiguousarray(qb[b]),
            "xk": np.ascontiguousarray(kb[b]),
            "xv": np.ascontiguousarray(vb[b]),
            "wq": np.ascontiguousarray(Wqb[:, sl]),
            "wk": np.ascontiguousarray(Wkb[:, sl]),
            "wv": np.ascontiguousarray(Wvb[:, sl]),
            "wo": np.ascontiguousarray(Wob[sl, :]),
            "bq": np.ascontiguousarray(bq[sl], dtype=f32),
            "bk": np.ascontiguousarray(bk[sl], dtype=f32),
            "bv": np.ascontiguousarray(bv[sl], dtype=f32),
        })
    return maps


def _gather(results, bo):
    out = np.empty((4, 2048, 1024), dtype=np.float32)
    bo32 = np.asarray(bo, dtype=np.float32)
    for b in range(4):
        out[b] = (results[2 * b]["out"].astype(np.float32)
                  + results[2 * b + 1]["out"].astype(np.float32) + bo32)
    return out


def kernel(q, k, v, Wq, bq, Wk, bk, Wv, bv, Wo, bo, _trace=False):
    nc = _get_nc()
    res = run_bass_kernel_spmd(
        nc, _in_maps(q, k, v, Wq, bq, Wk, bk, Wv, bv, Wo, bo),
        core_ids=list(range(8)), trace=_trace)
    out = _gather(res.results, bo)
    if _trace:
        kernel.last_results = res
    return out



# revision 8
# speedup vs baseline: 1.0218x; 1.0218x over previous
"""Bass/Tile multi-head attention kernel for TRN2.

Per-core problem (core c handles batch b=c//2, head-group g=c%2):
  inputs:  xq, xk, xv [S, DIN] bf16     (batch b slices of q/k/v, host-cast)
           wq, wk, wv [DIN, DC] bf16    (column slice for this head group)
           wo [DC, DOUT] bf16           (row slice)
           bq, bk, bv [DC] f32
  output:  out [S, DOUT] bf16  partial: host sums the two head-group
           partials per batch in f32 and adds bo.

Math (per head h of H local heads, depth=64):
  QT = (xq @ wq + bq).T        [DC, S]   bf16, d_core major
  KT = (xk @ wk + bk).T        [DC, S]   bf16
  V  = xv @ wv + bv            [S, DC]   bf16 (+ ones column -> V_aug)
  ST_h = KT_h.T @ QT_h         (64-deep contraction at partition base
                                (h%2)*64)
  E = exp(ST * 1/sqrt(depth))            (logits are O(6): no row-max pass)
  O_aug[q,:] = sum_k E[k,q] V_aug[k,:]   swapped AV: ex block stationary,
                                         [128,65] V chunk moving -> 65-row
                                         matmuls, full 128-deep contraction
  O accumulated in SBUF f32 (o_acc, DVE adds) so the AV stream can trail
  the ST stream by `lag` steps across q-tile boundaries.
  On = O[:, 0:64] / O[:, 64]             per-partition scalar on DVE
  OnT = XBAR SBUF->SBUF DMA transpose    (zero engine cost)
  out = OnT.T @ wo                       (bf16 x bf16 -> f32 psum)

Schedule: ONE global ST stream over (sqt, kg, h) in kg-major order feeds
the ACT exp stream back-to-back (ACT is the near-binding engine at
~266us; PE ~275us).  K chunks, V chunks, q-projections and
out-projections are injected between ST steps at tuned positions so the
PE load per step tracks ACT's 1038ns/step exp rate; the AV stream
trails globally by lag(j) (tapering) so V production slides late and
the drain tail stays short.

NOTE program order is load-bearing: every tile's writer must be EMITTED
before its first reader (the tile framework treats emission order as
happens-before; a reader emitted first reads garbage on hw).
"""

from collections import defaultdict
from contextlib import ExitStack

import concourse.mybir as mybir
from concourse import bacc
from concourse.tile import TileContext

F32 = mybir.dt.float32
F32R = mybir.dt.float32r
BF16 = mybir.dt.bfloat16
P = 128
EXP = mybir.ActivationFunctionType.Exp


def build_mha_core(S=2048, DIN=1024, DC=512, DOUT=1024, H=8, depth=64,
                   SQT=512, KG=2, num_devices=1, lag0=32, taper_start=128,
                   taper_div=2, lag_min=8, ex_bufs=36, qt_f32r=False,
                   kpos=(10, 26, 42), vpos=(26, 44, 60, 76)):
    assert DC == H * depth and DC % P == 0 and DIN % P == 0 and S % SQT == 0
    NKT = S // P          # key chunks of 128
    NDIN = DIN // P       # input-dim k-tiles
    NDO = DC // P         # d_core blocks
    NSQT = S // SQT       # attention q tiles
    NKG = NKT // KG       # kg groups per head
    NCH = S // 512        # 512-row x chunks
    NQC = SQT // P        # 128-query chunks per q tile
    NST = NSQT * NKG * H  # global st steps
    scale = 1.0 / float(depth) ** 0.5
    QTDT = F32R if qt_f32r else BF16

    nc = bacc.Bacc("TRN2", target_bir_lowering=False, debug=False,
                   num_devices=num_devices)
    xq = nc.dram_tensor("xq", [S, DIN], BF16, kind="ExternalInput")
    xk = nc.dram_tensor("xk", [S, DIN], BF16, kind="ExternalInput")
    xv = nc.dram_tensor("xv", [S, DIN], BF16, kind="ExternalInput")
    wq = nc.dram_tensor("wq", [DIN, DC], BF16, kind="ExternalInput")
    wk = nc.dram_tensor("wk", [DIN, DC], BF16, kind="ExternalInput")
    wv = nc.dram_tensor("wv", [DIN, DC], BF16, kind="ExternalInput")
    wo = nc.dram_tensor("wo", [DC, DOUT], BF16, kind="ExternalInput")
    bq = nc.dram_tensor("bq", [DC], F32, kind="ExternalInput")
    bk = nc.dram_tensor("bk", [DC], F32, kind="ExternalInput")
    bv = nc.dram_tensor("bv", [DC], F32, kind="ExternalInput")
    out = nc.dram_tensor("out", [S, DOUT], BF16, kind="ExternalOutput")

    with TileContext(nc) as tc, ExitStack() as ctx:
        const = ctx.enter_context(tc.tile_pool(name="const", bufs=1))
        wts = ctx.enter_context(tc.tile_pool(name="wts", bufs=1))
        kvpool = ctx.enter_context(tc.tile_pool(name="kv", bufs=1))
        xtpool = ctx.enter_context(tc.tile_pool(name="xt", bufs=3))
        qpool = ctx.enter_context(tc.tile_pool(name="qp", bufs=2))
        expool = ctx.enter_context(tc.tile_pool(name="ex", bufs=ex_bufs))
        oaccpool = ctx.enter_context(tc.tile_pool(name="oacc", bufs=1))
        otT_pool = ctx.enter_context(tc.tile_pool(name="otnt", bufs=2))
        otq_pool = ctx.enter_context(tc.tile_pool(name="otq", bufs=2))
        osbpool = ctx.enter_context(tc.tile_pool(name="osb", bufs=3))
        misc = ctx.enter_context(tc.tile_pool(name="misc", bufs=2))
        ps_st = ctx.enter_context(tc.tile_pool(name="ps_st", bufs=2, space="PSUM"))
        ps_ot = ctx.enter_context(tc.tile_pool(name="ps_ot", bufs=2, space="PSUM"))
        ps_gen = ctx.enter_context(tc.tile_pool(name="ps_gen", bufs=2, space="PSUM"))

        ones_f = const.tile([P, 1], F32)
        nc.vector.memset(ones_f[:], 1.0)

        bq_sb = const.tile([P, NDO], F32)
        bk_sb = const.tile([P, NDO], F32)
        bv_st = const.tile([1, DC], F32)
        bv_bc = const.tile([P, DC], F32)

        KT = kvpool.tile([P, NDO, S], BF16)
        V = kvpool.tile([P, NKT, H, depth + 1], BF16)
        nc.vector.tensor_copy(
            V[:, :, :, depth:depth + 1],
            ones_f[:, None, None, 0:1].to_broadcast((P, NKT, H, 1)))

        def load_weight(dram, kdim, ndim, tag, engs):
            # halves on parallel queues so both land ~together
            w = wts.tile([P, kdim // P, ndim], BF16, tag=tag, name=tag)
            half = kdim // P // 2
            for g, eng in enumerate(engs):
                eng.dma_start(
                    w[:, g * half:(g + 1) * half, :],
                    dram[g * half * P:(g + 1) * half * P, :]
                    .rearrange("(o p) n -> p o n", p=P))
            return w

        # ---- x transposition: 512 rows of xdram -> xt [P, NDIN, 512] ----
        xts = {}

        def emit_xt(key, xdram, c):
            xt = xtpool.tile([P, NDIN, 512], BF16, tag="xt", name="xt")
            hd = NDIN // 2
            for g in range(2):
                nc.sync.dma_start_transpose(
                    xt[:, g * hd:(g + 1) * hd, :],
                    xdram[c * 512:(c + 1) * 512,
                          g * hd * P:(g + 1) * hd * P])
            xts[key] = xt

        QTs = {}

        def qproj_block(sqt, do):
            if do == 0:
                QTs[sqt] = qpool.tile([P, NDO, SQT], QTDT, tag="qt",
                                      name="qt")
            xt, QT = xts[("q", sqt)], QTs[sqt]
            ps = ps_gen.tile([P, 512], F32, tag="gen", name="psq")
            for kt in range(NDIN):
                nc.tensor.matmul(
                    ps[:, :SQT], wqr[:, kt, do * P:(do + 1) * P],
                    xt[:, kt, :], start=(kt == 0), stop=(kt == NDIN - 1))
            nc.vector.tensor_scalar_add(QT[:, do, :], ps[:, :SQT],
                                        bq_sb[:, do:do + 1])

        def k_chunk(c):
            xt = xts[("k", c)]
            for do in range(NDO):
                ps = ps_gen.tile([P, 512], F32, tag="gen", name="psk")
                for kt in range(NDIN):
                    nc.tensor.matmul(
                        ps[:], wkr[:, kt, do * P:(do + 1) * P],
                        xt[:, kt, :], start=(kt == 0),
                        stop=(kt == NDIN - 1))
                nc.vector.tensor_scalar_add(
                    KT[:, do, c * 512:(c + 1) * 512], ps[:],
                    bk_sb[:, do:do + 1])

        def v_chunk(c):
            xt = xts[("v", c)]
            for sc in range(4):
                ps = ps_gen.tile([P, 512], F32, tag="gen", name="psv")
                for kt in range(NDIN):
                    nc.tensor.matmul(
                        ps[:], xt[:, kt, sc * P:(sc + 1) * P],
                        wvr[:, kt, :], start=(kt == 0),
                        stop=(kt == NDIN - 1))
                nc.vector.tensor_tensor(
                    V[:, c * 4 + sc, :, 0:depth],
                    ps[:].rearrange("p (h d) -> p h d", h=H),
                    bv_bc[:].rearrange("p (h d) -> p h d", h=H),
                    mybir.AluOpType.add)

        # ---- attention streams ----
        exs, oaccs, OTnTs, otqs = {}, {}, {}, {}

        def st_step(s, kg, h):
            p0, blk = (h % 2) * 64, h // 2
            QT = QTs[s]
            st = ps_st.tile([P, KG, 512], F32, tag="st", name="st")
            for j in range(KG):
                kt = kg * KG + j
                nc.tensor.matmul(
                    st[:, j], KT[p0:p0 + 64, blk, kt * P:(kt + 1) * P],
                    QT[p0:p0 + 64, blk, :], start=True, stop=True)
            ex = expool.tile([P, KG, 512], BF16, tag="ex", name="ex")
            exs[(s, kg, h)] = ex
            nc.scalar.activation(ex[:], st[:], EXP, scale=scale)

        def norm_head(s, h):
            pair, p0 = h // 2, (h % 2) * 64
            fine = (s == NSQT - 1 and h == H - 1)
            if h == 0:
                OTnTs[s] = [otT_pool.tile([P, SQT], BF16, tag=f"otnt{b}",
                                          name="otnt") for b in range(NDO)]
            if h % 2 == 0:
                otqs[(s, pair)] = otq_pool.tile([P, NQC, P], BF16,
                                                tag="otq", name="otq")
            q_tile = otqs[(s, pair)]
            oa = oaccs[s][:, h]
            if fine:
                # last head of the last tile: per-qc chain so each
                # out-projection chunk un-gates as early as possible
                for qc in range(NQC):
                    rec = misc.tile([P, 1, 1], F32, tag="recf", name="recf")
                    nc.vector.reciprocal(
                        rec[:], oa[:, qc:qc + 1, depth:depth + 1])
                    nc.vector.tensor_tensor(
                        q_tile[:, qc:qc + 1, p0:p0 + depth],
                        oa[:, qc:qc + 1, 0:depth],
                        rec[:, :, 0:1].to_broadcast((P, 1, depth)),
                        mybir.AluOpType.mult)
                    nc.sync.dma_start_transpose(
                        OTnTs[s][pair][:, qc * P:(qc + 1) * P],
                        q_tile[:, qc, :])
                return
            rec = misc.tile([P, NQC, 1], F32, tag="rec", name="rec")
            nc.vector.reciprocal(rec[:], oa[:, :, depth:depth + 1])
            nc.vector.tensor_tensor(
                q_tile[:, :, p0:p0 + depth], oa[:, :, 0:depth],
                rec[:, :, 0:1].to_broadcast((P, NQC, depth)),
                mybir.AluOpType.mult)
            if h % 2 == 1:
                for qc in range(NQC):
                    nc.sync.dma_start_transpose(
                        OTnTs[s][pair][:, qc * P:(qc + 1) * P],
                        q_tile[:, qc, :])

        def av_step(s, kg, h):
            if kg == 0 and h == 0:
                oaccs[s] = oaccpool.tile([P, H, NQC, depth + 1], F32,
                                         tag="oacc", name="oacc")
            ex = exs.pop((s, kg, h))
            ps = ps_ot.tile([P, NQC, P], F32, tag="ot", name="ot")
            for j in range(KG):
                kt = kg * KG + j
                for qc in range(NQC):
                    # start/stop once per BANK: start_tensor_calc marks the
                    # whole 2KB zero region pending-zero, so the first write
                    # of each qc sub-region auto-overwrites
                    nc.tensor.matmul(
                        ps[:, qc, 0:depth + 1],
                        ex[:, j, qc * P:(qc + 1) * P],
                        V[:, kt, h, :],
                        start=(j == 0 and qc == 0),
                        stop=(j == KG - 1 and qc == NQC - 1))
            oa = oaccs[s][:, h]
            if kg == 0:
                nc.vector.tensor_copy(oa[:, :, :], ps[:, :, 0:depth + 1])
            else:
                nc.vector.tensor_tensor(oa[:, :, :], oa[:, :, :],
                                        ps[:, :, 0:depth + 1],
                                        mybir.AluOpType.add)
            if kg == NKG - 1:
                norm_head(s, h)

        def do_oproj_sc(s, sc, copy_act=False, out_sync=False):
            OTnT = OTnTs[s]
            osb = osbpool.tile([P, DOUT], BF16, tag="osb", name="osb")
            r0 = s * SQT + sc * P
            for do in range(DOUT // 512):
                ps = ps_gen.tile([P, 512], F32, tag="gen", name="pso")
                for i in range(NDO):
                    nc.tensor.matmul(
                        ps[:], OTnT[i][:, sc * P:(sc + 1) * P],
                        wor[:, i, do * 512:(do + 1) * 512],
                        start=(i == 0), stop=(i == NDO - 1))
                if copy_act:
                    nc.scalar.copy(osb[:, do * 512:(do + 1) * 512], ps[:])
                else:
                    nc.vector.tensor_copy(
                        osb[:, do * 512:(do + 1) * 512], ps[:])
                (nc.sync if out_sync else nc.gpsimd).dma_start(
                    out[r0:r0 + P, do * 512:(do + 1) * 512],
                    osb[:, do * 512:(do + 1) * 512])

        # ---- injection schedule ----
        inject = defaultdict(list)
        for i, c in zip(kpos, (1, 2, 3)):
            inject[max(0, i - 8)].append(lambda c=c: emit_xt(("k", c), xk, c))
            inject[i].append(lambda c=c: k_chunk(c))
        inject[12].append(lambda: globals_wv())
        for i, c in zip(vpos, (0, 1, 2, 3)):
            inject[max(0, i - 8)].append(lambda c=c: emit_xt(("v", c), xv, c))
            inject[i].append(lambda c=c: v_chunk(c))
        inject[66].append(lambda: globals_wo())
        for s1 in range(1, NSQT):
            base = 64 * (s1 - 1)
            inject[base + 40].append(lambda s1=s1: emit_xt(("q", s1), xq, s1))
            for b in range(NDO):
                inject[base + 46 + 4 * b].append(
                    lambda s1=s1, b=b: qproj_block(s1, b))
        # out-projections: op(0) in s2, op(1)+op(2) in s3, op(3) in drain
        for sc in range(NQC):
            inject[130 + 4 * sc].append(lambda sc=sc: do_oproj_sc(0, sc))
            inject[194 + 4 * sc].append(lambda sc=sc: do_oproj_sc(1, sc))
            inject[222 + 4 * sc].append(lambda sc=sc: do_oproj_sc(2, sc))

        wvr = wor = None

        def globals_wv():
            nonlocal wvr
            wvr = load_weight(wv, DIN, DC, "wv", (nc.sync, nc.gpsimd))

        def globals_wo():
            nonlocal wor
            wor = load_weight(wo, DC, DOUT, "wo", (nc.sync, nc.gpsimd))

        def lag(j):
            if j < taper_start:
                return lag0
            return max(lag_min, lag0 - (j - taper_start) // taper_div)

        # ---- pre-loop: biases, wq/wk, xqT0/xkT0, qproj(0), K chunk 0 ----
        nc.gpsimd.dma_start(bq_sb[:], bq[:].rearrange("(o p) -> p o", p=P))
        emit_xt(("q", 0), xq, 0)
        wqr = load_weight(wq, DIN, DC, "wq", (nc.scalar, nc.scalar))
        emit_xt(("k", 0), xk, 0)
        wkr = load_weight(wk, DIN, DC, "wk", (nc.scalar, nc.gpsimd))
        nc.gpsimd.dma_start(bk_sb[:], bk[:].rearrange("(o p) -> p o", p=P))
        nc.gpsimd.dma_start(bv_st[0:1, :], bv[:][None, :])
        nc.gpsimd.partition_broadcast(bv_bc[:], bv_st[0:1, :])
        for b in range(NDO):
            qproj_block(0, b)
        k_chunk(0)

        # ---- global ST stream with trailing AV stream ----
        av_j = [0]

        def drain_avs(upto_pos):
            while av_j[0] < NST and av_j[0] + lag(av_j[0]) <= upto_pos:
                j = av_j[0]
                s, r = divmod(j, NKG * H)
                kg, h = divmod(r, H)
                av_step(s, kg, h)
                av_j[0] += 1

        for i in range(NST):
            for fn in inject.get(i, ()):
                fn()
            s, r = divmod(i, NKG * H)
            kg, h = divmod(r, H)
            st_step(s, kg, h)
            drain_avs(i)
        drain_avs(NST + lag0 + 1)

        for sc in range(NQC):
            do_oproj_sc(NSQT - 1, sc, copy_act=True, out_sync=True)

    nc.compile()
    return nc


# ---------------------------------------------------------------------------
# Host-side wrapper: shard across 8 NeuronCores, run SPMD, gather.
# Core c handles batch b = c // 2 and head-group g = c % 2 (8 of 16 heads,
# i.e. columns [g*512, (g+1)*512) of Wq/Wk/Wv and rows of Wo).
# ---------------------------------------------------------------------------

import ml_dtypes
import numpy as np

from concourse.bass_utils import run_bass_kernel_spmd

_BF16 = ml_dtypes.bfloat16

_NC = None


def _get_nc():
    global _NC
    if _NC is None:
        _NC = build_mha_core(S=2048, DIN=1024, DC=512, DOUT=1024, H=8,
                             depth=64, num_devices=8)
    return _NC


def _in_maps(q, k, v, Wq, bq, Wk, bk, Wv, bv, Wo, bo):
    f32 = np.float32
    qb = np.asarray(q, dtype=_BF16)
    kb = np.asarray(k, dtype=_BF16)
    vb = np.asarray(v, dtype=_BF16)
    Wqb = np.asarray(Wq, dtype=_BF16)
    Wkb = np.asarray(Wk, dtype=_BF16)
    Wvb = np.asarray(Wv, dtype=_BF16)
    Wob = np.asarray(Wo, dtype=_BF16)
    maps = []
    for c in range(8):
        b, g = c // 2, c % 2
        sl = slice(g * 512, (g + 1) * 512)
        maps.append({
            "xq": np.ascontiguousarray(qb[b]),
            "xk": np.ascontiguousarray(kb[b]),
            "xv": np.ascontiguousarray(vb[b]),
            "wq": np.ascontiguousarray(Wqb[:, sl]),
            "wk": np.ascontiguousarray(Wkb[:, sl]),
            "wv": np.ascontiguousarray(Wvb[:, sl]),
            "wo": np.ascontiguousarray(Wob[sl, :]),
            "bq": np.ascontiguousarray(bq[sl], dtype=f32),
            "bk": np.ascontiguousarray(bk[sl], dtype=f32),
            "bv": np.ascontiguousarray(bv[sl], dtype=f32),
        })
    return maps


def _gather(results, bo):
    out = np.empty((4, 2048, 1024), dtype=np.float32)
    bo32 = np.asarray(bo, dtype=np.float32)
    for b in range(4):
        out[b] = (results[2 * b]["out"].astype(np.float32)
                  + results[2 * b + 1]["out"].astype(np.float32) + bo32)
    return out


def kernel(q, k, v, Wq, bq, Wk, bk, Wv, bv, Wo, bo, _trace=False):
    nc = _get_nc()
    res = run_bass_kernel_spmd(
        nc, _in_maps(q, k, v, Wq, bq, Wk, bk, Wv, bv, Wo, bo),
        core_ids=list(range(8)), trace=_trace)
    out = _gather(res.results, bo)
    if _trace:
        kernel.last_results = res
    return out


# revision 17
# speedup vs baseline: 1.0279x; 1.0060x over previous
"""Bass/Tile multi-head attention kernel for TRN2.

Per-core problem (core c handles batch b=c//2, head-group g=c%2):
  inputs:  xq, xk, xv [S, DIN] bf16     (batch b slices of q/k/v, host-cast)
           wq, wk, wv [DIN, DC] bf16    (column slice for this head group)
           wo [DC, DOUT] bf16           (row slice)
           bq, bk, bv [DC] f32
  output:  out [S, DOUT] bf16  partial: host sums the two head-group
           partials per batch in f32 and adds bo.

Math (per head h of H local heads, depth=64):
  QT = (xq @ wq + bq).T        [DC, S]   bf16, d_core major
  KT = (xk @ wk + bk).T        [DC, S]   bf16
  V  = xv @ wv + bv            [S, DC]   bf16 (+ ones column -> V_aug)
  ST_h = KT_h.T @ QT_h         (64-deep contraction at partition base
                                (h%2)*64)
  E = exp(ST * 1/sqrt(depth))            (logits are O(6): no row-max pass)
  O_aug[q,:] = sum_k E[k,q] V_aug[k,:]   swapped AV: ex block stationary,
                                         [128,65] V chunk moving -> 65-row
                                         matmuls, full 128-deep contraction
  O accumulated in SBUF f32 (o_acc, DVE adds) so the AV stream can trail
  the ST stream by `lag` steps across q-tile boundaries.
  On = O[:, 0:64] / O[:, 64]             per-partition scalar on DVE
  OnT = XBAR SBUF->SBUF DMA transpose    (zero engine cost)
  out = OnT.T @ wo                       (bf16 x bf16 -> f32 psum)

Schedule: ONE global ST stream over (sqt, kg, h) in kg-major order feeds
the ACT exp stream back-to-back (ACT is the near-binding engine at
~266us; PE ~275us).  K chunks, V chunks, q-projections and
out-projections are injected between ST steps at tuned positions so the
PE load per step tracks ACT's 1038ns/step exp rate; the AV stream
trails globally by lag(j) (tapering) so V production slides late and
the drain tail stays short.

NOTE program order is load-bearing: every tile's writer must be EMITTED
before its first reader (the tile framework treats emission order as
happens-before; a reader emitted first reads garbage on hw).
"""

from collections import defaultdict
from contextlib import ExitStack

import concourse.mybir as mybir
from concourse import bacc
from concourse.tile import TileContext

F32 = mybir.dt.float32
F32R = mybir.dt.float32r
BF16 = mybir.dt.bfloat16
P = 128
EXP = mybir.ActivationFunctionType.Exp


def build_mha_core(S=2048, DIN=1024, DC=512, DOUT=1024, H=8, depth=64,
                   SQT=512, KG=2, num_devices=1, lag0=38, taper_start=120,
                   taper_div=2, lag_min=4, ex_bufs=40, qt_f32r=False,
                   kpos=(12, 28, 44), kxpos=(6, 22), vpos=(36, 52, 66, 82),
                   vxpos=(28, 44, 58, 74), op2base=202):
    assert DC == H * depth and DC % P == 0 and DIN % P == 0 and S % SQT == 0
    NKT = S // P          # key chunks of 128
    NDIN = DIN // P       # input-dim k-tiles
    NDO = DC // P         # d_core blocks
    NSQT = S // SQT       # attention q tiles
    NKG = NKT // KG       # kg groups per head
    NCH = S // 512        # 512-row x chunks
    NQC = SQT // P        # 128-query chunks per q tile
    NST = NSQT * NKG * H  # global st steps
    scale = 1.0 / float(depth) ** 0.5
    QTDT = F32R if qt_f32r else BF16

    nc = bacc.Bacc("TRN2", target_bir_lowering=False, debug=False,
                   num_devices=num_devices)
    xq = nc.dram_tensor("xq", [S, DIN], BF16, kind="ExternalInput")
    xk = nc.dram_tensor("xk", [S, DIN], BF16, kind="ExternalInput")
    xv = nc.dram_tensor("xv", [S, DIN], BF16, kind="ExternalInput")
    wq = nc.dram_tensor("wq", [DIN, DC], BF16, kind="ExternalInput")
    wk = nc.dram_tensor("wk", [DIN, DC], BF16, kind="ExternalInput")
    wv = nc.dram_tensor("wv", [DIN, DC], BF16, kind="ExternalInput")
    wo = nc.dram_tensor("wo", [DC, DOUT], BF16, kind="ExternalInput")
    bq = nc.dram_tensor("bq", [DC], F32, kind="ExternalInput")
    bk = nc.dram_tensor("bk", [DC], F32, kind="ExternalInput")
    bv = nc.dram_tensor("bv", [DC], F32, kind="ExternalInput")
    out = nc.dram_tensor("out", [S, DOUT], BF16, kind="ExternalOutput")

    with TileContext(nc) as tc, ExitStack() as ctx:
        const = ctx.enter_context(tc.tile_pool(name="const", bufs=1))
        wts = ctx.enter_context(tc.tile_pool(name="wts", bufs=1))
        kvpool = ctx.enter_context(tc.tile_pool(name="kv", bufs=1))
        xqpool = ctx.enter_context(tc.tile_pool(name="xq", bufs=2))
        xkvpool = ctx.enter_context(tc.tile_pool(name="xkv", bufs=2))
        qpool = ctx.enter_context(tc.tile_pool(name="qp", bufs=2))
        expool = ctx.enter_context(tc.tile_pool(name="ex", bufs=ex_bufs))
        oaccpool = ctx.enter_context(tc.tile_pool(name="oacc", bufs=1))
        otT_pool = ctx.enter_context(tc.tile_pool(name="otnt", bufs=2))
        otq_pool = ctx.enter_context(tc.tile_pool(name="otq", bufs=2))
        osbpool = ctx.enter_context(tc.tile_pool(name="osb", bufs=2))
        misc = ctx.enter_context(tc.tile_pool(name="misc", bufs=2))
        ps_st = ctx.enter_context(tc.tile_pool(name="ps_st", bufs=2, space="PSUM"))
        ps_ot = ctx.enter_context(tc.tile_pool(name="ps_ot", bufs=2, space="PSUM"))
        ps_gen = ctx.enter_context(tc.tile_pool(name="ps_gen", bufs=2, space="PSUM"))

        ones_f = const.tile([P, 1], F32)
        nc.vector.memset(ones_f[:], 1.0)

        bq_sb = const.tile([P, NDO], F32)
        bk_sb = const.tile([P, NDO], F32)
        bv_st = const.tile([1, DC], F32)
        bv_bc = const.tile([P, DC], F32)

        KT = kvpool.tile([P, NDO, S], BF16)
        V = kvpool.tile([P, NKT, H, depth + 1], BF16)
        nc.vector.tensor_copy(
            V[:, :, :, depth:depth + 1],
            ones_f[:, None, None, 0:1].to_broadcast((P, NKT, H, 1)))

        def load_weight(dram, kdim, ndim, tag, engs):
            # halves on parallel queues so both land ~together
            w = wts.tile([P, kdim // P, ndim], BF16, tag=tag, name=tag)
            half = kdim // P // 2
            for g, eng in enumerate(engs):
                eng.dma_start(
                    w[:, g * half:(g + 1) * half, :],
                    dram[g * half * P:(g + 1) * half * P, :]
                    .rearrange("(o p) n -> p o n", p=P))
            return w

        # ---- x transposition: 512 rows of xdram -> xt [P, NDIN, 512] ----
        # separate pools for q vs k/v so ring-WAR waits of one stream never
        # SEQ-block the other stream's transposes behind them in the queue
        xts = {}

        def emit_xt(key, xdram, c):
            pool = xqpool if key[0] == "q" else xkvpool
            eng = nc.sync
            xt = pool.tile([P, NDIN, 512], BF16, tag="xt", name="xt")
            hd = NDIN // 2
            for g in range(2):
                eng.dma_start_transpose(
                    xt[:, g * hd:(g + 1) * hd, :],
                    xdram[c * 512:(c + 1) * 512,
                          g * hd * P:(g + 1) * hd * P])
            xts[key] = xt

        QTs = {}

        def qproj_block(sqt, do):
            if do == 0:
                QTs[sqt] = qpool.tile([P, NDO, SQT], QTDT, tag="qt",
                                      name="qt")
            xt, QT = xts[("q", sqt)], QTs[sqt]
            ps = ps_gen.tile([P, 512], F32, tag="gen", name="psq")
            for kt in range(NDIN):
                nc.tensor.matmul(
                    ps[:, :SQT], wqr[:, kt, do * P:(do + 1) * P],
                    xt[:, kt, :], start=(kt == 0), stop=(kt == NDIN - 1))
            nc.vector.tensor_scalar_add(QT[:, do, :], ps[:, :SQT],
                                        bq_sb[:, do:do + 1])

        def k_block(c, do):
            xt = xts[("k", c)]
            ps = ps_gen.tile([P, 512], F32, tag="gen", name="psk")
            for kt in range(NDIN):
                nc.tensor.matmul(
                    ps[:], wkr[:, kt, do * P:(do + 1) * P],
                    xt[:, kt, :], start=(kt == 0),
                    stop=(kt == NDIN - 1))
            nc.vector.tensor_scalar_add(
                KT[:, do, c * 512:(c + 1) * 512], ps[:],
                bk_sb[:, do:do + 1])

        def k_chunk(c):
            for do in range(NDO):
                k_block(c, do)

        def v_chunk(c):
            xt = xts[("v", c)]
            for sc in range(4):
                ps = ps_gen.tile([P, 512], F32, tag="gen", name="psv")
                for kt in range(NDIN):
                    nc.tensor.matmul(
                        ps[:], xt[:, kt, sc * P:(sc + 1) * P],
                        wvr[:, kt, :], start=(kt == 0),
                        stop=(kt == NDIN - 1))
                nc.vector.tensor_tensor(
                    V[:, c * 4 + sc, :, 0:depth],
                    ps[:].rearrange("p (h d) -> p h d", h=H),
                    bv_bc[:].rearrange("p (h d) -> p h d", h=H),
                    mybir.AluOpType.add)

        # ---- attention streams ----
        exs, oaccs, OTnTs, otqs = {}, {}, {}, {}

        def st_step(s, kg, h):
            p0, blk = (h % 2) * 64, h // 2
            QT = QTs[s]
            st = ps_st.tile([P, KG, 512], F32, tag="st", name="st")
            for j in range(KG):
                kt = kg * KG + j
                nc.tensor.matmul(
                    st[:, j], KT[p0:p0 + 64, blk, kt * P:(kt + 1) * P],
                    QT[p0:p0 + 64, blk, :], start=True, stop=True)
            ex = expool.tile([P, KG, 512], BF16, tag="ex", name="ex")
            exs[(s, kg, h)] = ex
            nc.scalar.activation(ex[:], st[:], EXP, scale=scale)

        def norm_head(s, h):
            pair, p0 = h // 2, (h % 2) * 64
            fine = (s == NSQT - 1 and h == H - 1)
            if h == 0:
                OTnTs[s] = [otT_pool.tile([P, SQT], BF16, tag=f"otnt{b}",
                                          name="otnt") for b in range(NDO)]
            if h % 2 == 0:
                otqs[(s, pair)] = otq_pool.tile([P, NQC, P], BF16,
                                                tag="otq", name="otq")
            q_tile = otqs[(s, pair)]
            oa = oaccs[s][:, h]
            if fine:
                # last head of the last tile: per-qc chain so each
                # out-projection chunk un-gates as early as possible
                for qc in range(NQC):
                    rec = misc.tile([P, 1, 1], F32, tag="recf", name="recf")
                    nc.vector.reciprocal(
                        rec[:], oa[:, qc:qc + 1, depth:depth + 1])
                    nc.vector.tensor_tensor(
                        q_tile[:, qc:qc + 1, p0:p0 + depth],
                        oa[:, qc:qc + 1, 0:depth],
                        rec[:, :, 0:1].to_broadcast((P, 1, depth)),
                        mybir.AluOpType.mult)
                    nc.sync.dma_start_transpose(
                        OTnTs[s][pair][:, qc * P:(qc + 1) * P],
                        q_tile[:, qc, :])
                return
            rec = misc.tile([P, NQC, 1], F32, tag="rec", name="rec")
            nc.vector.reciprocal(rec[:], oa[:, :, depth:depth + 1])
            nc.vector.tensor_tensor(
                q_tile[:, :, p0:p0 + depth], oa[:, :, 0:depth],
                rec[:, :, 0:1].to_broadcast((P, NQC, depth)),
                mybir.AluOpType.mult)
            if h % 2 == 1:
                for qc in range(NQC):
                    nc.sync.dma_start_transpose(
                        OTnTs[s][pair][:, qc * P:(qc + 1) * P],
                        q_tile[:, qc, :])

        def av_step(s, kg, h):
            if kg == 0 and h == 0:
                oaccs[s] = oaccpool.tile([P, H, NQC, depth + 1], F32,
                                         tag="oacc", name="oacc")
            ex = exs.pop((s, kg, h))
            ps = ps_ot.tile([P, NQC, P], F32, tag="ot", name="ot")
            for j in range(KG):
                kt = kg * KG + j
                for qc in range(NQC):
                    # start/stop once per BANK: start_tensor_calc marks the
                    # whole 2KB zero region pending-zero, so the first write
                    # of each qc sub-region auto-overwrites
                    nc.tensor.matmul(
                        ps[:, qc, 0:depth + 1],
                        ex[:, j, qc * P:(qc + 1) * P],
                        V[:, kt, h, :],
                        start=(j == 0 and qc == 0),
                        stop=(j == KG - 1 and qc == NQC - 1))
            oa = oaccs[s][:, h]
            if kg == 0:
                nc.vector.tensor_copy(oa[:, :, :], ps[:, :, 0:depth + 1])
            else:
                nc.vector.tensor_tensor(oa[:, :, :], oa[:, :, :],
                                        ps[:, :, 0:depth + 1],
                                        mybir.AluOpType.add)
            if kg == NKG - 1:
                norm_head(s, h)

        def do_oproj_sc(s, sc, copy_act=False, out_sync=False):
            OTnT = OTnTs[s]
            osb = osbpool.tile([P, DOUT], BF16, tag="osb", name="osb")
            r0 = s * SQT + sc * P
            for do in range(DOUT // 512):
                ps = ps_gen.tile([P, 512], F32, tag="gen", name="pso")
                for i in range(NDO):
                    nc.tensor.matmul(
                        ps[:], OTnT[i][:, sc * P:(sc + 1) * P],
                        wor[:, i, do * 512:(do + 1) * 512],
                        start=(i == 0), stop=(i == NDO - 1))
                if copy_act:
                    nc.scalar.copy(osb[:, do * 512:(do + 1) * 512], ps[:])
                else:
                    nc.vector.tensor_copy(
                        osb[:, do * 512:(do + 1) * 512], ps[:])
                (nc.sync if out_sync else nc.gpsimd).dma_start(
                    out[r0:r0 + P, do * 512:(do + 1) * 512],
                    osb[:, do * 512:(do + 1) * 512])

        # ---- injection schedule ----
        inject = defaultdict(list)
        # qproj(0) and K chunk 0 interleave with the first st steps:
        # st(kg0, h) needs QT blk h//2 and KT chunk0 blk h//2 only, so the
        # first exp fires ~10us earlier than an up-front emission
        for b in range(NDO):
            inject[2 * b].append(lambda b=b: qproj_block(0, b))
            inject[2 * b].append(lambda b=b: k_block(0, b))
        for i, c in zip(kxpos, (2, 3)):
            inject[i].append(lambda c=c: emit_xt(("k", c), xk, c))
        for i, c in zip(kpos, (1, 2, 3)):
            inject[i].append(lambda c=c: k_chunk(c))
        inject[10].append(lambda: globals_wv())
        for i, c in zip(vxpos, (0, 1, 2, 3)):
            inject[i].append(lambda c=c: emit_xt(("v", c), xv, c))
        for i, c in zip(vpos, (0, 1, 2, 3)):
            inject[i].append(lambda c=c: v_chunk(c))
        inject[60].append(lambda: globals_wo())
        for s1 in range(1, NSQT):
            base = 64 * (s1 - 1)
            inject[base + 38].append(lambda s1=s1: emit_xt(("q", s1), xq, s1))
            for b in range(NDO):
                inject[base + 46 + 4 * b].append(
                    lambda s1=s1, b=b: qproj_block(s1, b))
        # out-projections: op(0) in s2, op(1)+op(2) in s3, op(3) in drain
        for sc in range(NQC):
            inject[130 + 4 * sc].append(lambda sc=sc: do_oproj_sc(0, sc))
            inject[194 + 4 * sc].append(lambda sc=sc: do_oproj_sc(1, sc))
            inject[op2base + 4 * sc].append(lambda sc=sc: do_oproj_sc(2, sc))

        wvr = wor = None

        def globals_wv():
            nonlocal wvr
            wvr = load_weight(wv, DIN, DC, "wv", (nc.sync, nc.gpsimd))

        def globals_wo():
            # wo reuses wk's slot (tag "wkwo", bufs=1): wk's last reader
            # (K chunk 3) is emitted before this, so the WAR dep is clean
            nonlocal wor
            wor = load_weight(wo, DC, DOUT, "wkwo", (nc.sync, nc.gpsimd))

        def lag(j):
            if j < taper_start:
                return lag0
            return max(lag_min, lag0 - (j - taper_start) // taper_div)

        # ---- pre-loop: DMAs only (biases, wq/wk, xqT0, xkT0/1) ----
        nc.gpsimd.dma_start(bq_sb[:], bq[:].rearrange("(o p) -> p o", p=P))
        emit_xt(("q", 0), xq, 0)
        wqr = load_weight(wq, DIN, DC, "wq", (nc.scalar, nc.scalar))
        emit_xt(("k", 0), xk, 0)
        wkr = load_weight(wk, DIN, DC, "wkwo", (nc.scalar, nc.gpsimd))
        emit_xt(("k", 1), xk, 1)
        nc.gpsimd.dma_start(bk_sb[:], bk[:].rearrange("(o p) -> p o", p=P))
        nc.gpsimd.dma_start(bv_st[0:1, :], bv[:][None, :])
        nc.gpsimd.partition_broadcast(bv_bc[:], bv_st[0:1, :])

        # ---- global ST stream with trailing AV stream ----
        av_j = [0]

        def drain_avs(upto_pos):
            while av_j[0] < NST and av_j[0] + lag(av_j[0]) <= upto_pos:
                j = av_j[0]
                s, r = divmod(j, NKG * H)
                kg, h = divmod(r, H)
                av_step(s, kg, h)
                av_j[0] += 1

        for i in range(NST):
            for fn in inject.get(i, ()):
                fn()
            s, r = divmod(i, NKG * H)
            kg, h = divmod(r, H)
            st_step(s, kg, h)
            drain_avs(i)
        drain_avs(NST + lag0 + 1)

        for sc in range(NQC):
            do_oproj_sc(NSQT - 1, sc, copy_act=True, out_sync=True)

    nc.compile()
    return nc


# ---------------------------------------------------------------------------
# Host-side wrapper: shard across 8 NeuronCores, run SPMD, gather.
# Core c handles batch b = c // 2 and head-group g = c % 2 (8 of 16 heads,
# i.e. columns [g*512, (g+1)*512) of Wq/Wk/Wv and rows of Wo).
# ---------------------------------------------------------------------------

import ml_dtypes
import numpy as np

from concourse.bass_utils import run_bass_kernel_spmd

_BF16 = ml_dtypes.bfloat16

_NC = None


def _get_nc():
    global _NC
    if _NC is None:
        _NC = build_mha_core(S=2048, DIN=1024, DC=512, DOUT=1024, H=8,
                             depth=64, num_devices=8)
    return _NC


def _in_maps(q, k, v, Wq, bq, Wk, bk, Wv, bv, Wo, bo):
    f32 = np.float32
    qb = np.asarray(q, dtype=_BF16)
    kb = np.asarray(k, dtype=_BF16)
    vb = np.asarray(v, dtype=_BF16)
    Wqb = np.asarray(Wq, dtype=_BF16)
    Wkb = np.asarray(Wk, dtype=_BF16)
    Wvb = np.asarray(Wv, dtype=_BF16)
    Wob = np.asarray(Wo, dtype=_BF16)
    maps = []
    for c in range(8):
        b, g = c // 2, c % 2
        sl = slice(g * 512, (g + 1) * 512)
        maps.append({
            "xq": np.ascontiguousarray(qb[b]),
            "xk": np.ascontiguousarray(kb[b]),
            "xv": np.ascontiguousarray(vb[b]),
            "wq": np.ascontiguousarray(Wqb[:, sl]),
            "wk": np.ascontiguousarray(Wkb[:, sl]),
            "wv": np.ascontiguousarray(Wvb[:, sl]),
            "wo": np.ascontiguousarray(Wob[sl, :]),
            "bq": np.ascontiguousarray(bq[sl], dtype=f32),
            "bk": np.ascontiguousarray(bk[sl], dtype=f32),
            "bv": np.ascontiguousarray(bv[sl], dtype=f32),
        })
    return maps


def _gather(results, bo):
    out = np.empty((4, 2048, 1024), dtype=np.float32)
    bo32 = np.asarray(bo, dtype=np.float32)
    for b in range(4):
        out[b] = (results[2 * b]["out"].astype(np.float32)
                  + results[2 * b + 1]["out"].astype(np.float32) + bo32)
    return out


def kernel(q, k, v, Wq, bq, Wk, bk, Wv, bv, Wo, bo, _trace=False):
    nc = _get_nc()
    res = run_bass_kernel_spmd(
        nc, _in_maps(q, k, v, Wq, bq, Wk, bk, Wv, bv, Wo, bo),
        core_ids=list(range(8)), trace=_trace)
    out = _gather(res.results, bo)
    if _trace:
        kernel.last_results = res
    return out
